# revision 12
# baseline (speedup 1.0000x reference)
"""Trainium2 Bass kernel for fused MHA with q/k std-normalization.

Reference computation (per batch b, head h):
    q,k,v = x[b].T @ Wq/Wk/Wv          [T, 64] each
    q = (q - mean_e) / (std_e(ddof=1) + 1e-5)   (same for k)
    attn = softmax(q @ k.T / 8)
    out[b, h*64:(h+1)*64, :] = (attn @ v).T

Sharding: 8 cores = 4 batches x 2 half-head-groups. Core c handles batch
c//2, heads (c%2)*8 .. (c%2)*8+8. Fully head-independent, no collectives.

Per-core pipeline, structured so the ACT-bound softmax exp (the largest
irreducible per-engine cost, ~266us busy) overlaps the q-projection and
the attention PE work instead of idling during a separate QKV phase:

  Pass A (k/v projection, 16 t-tiles): per tile one psum tile
    [128t, (k|v, h, e)] accumulated over 8 d-chunks (two N=512 f32r
    matmuls per chunk). Per-head bn_stats/bn_aggr on the k block read
    PSUM directly; one batched ACT sqrt per tile covers 8 sigmas; DVE
    computes eps+reciprocal and -mean*inv; khat = k*inv + (-mean*inv)
    runs on the otherwise-idle ACT (Identity with per-partition
    scale/bias APs) into bf16 kstg, PE-transposed (1 cyc/row) into kTi;
    all 8 heads' v evacuated bf16 with a single DVE copy into vt
    ([s 128, 64 v | 1]; the ones column feeds the softmax denominator
    through the attention*V matmul).

  Pass B (q projection fused with attention): the next strip's four
    q-tiles are prepared interleaved between attention units: units
    h=0..3 are followed by one q-tile projection each (per-head
    bn_stats from PSUM, raw q parked bf16 in SBUF to free the psum
    bank), after h=3 ONE batched ACT sqrt covers all 4 tiles x 8 heads
    (2 act-table switches per strip total), and units h=4..7 are
    followed by one tile's normalize (DVE 2-byte fast mode from bf16
    raw q, duplicated into both column halves) + PE transpose +
    evacuation into qTd.

    Attention unit (h, st): scores^T [s,t] = kT-chunk.T @ qT-strip
    (K=64 bf16, chunk pairs in separate PE row-groups via
    tile_position); exp on ACT with scale=1/8 folded in (|q||k|/8 <= 8
    so max-subtraction-free softmax is fp32-safe) producing bf16 probs;
    attention*V (bf16) accumulates op [65, 512] over 16 s-chunks (row
    64 = sum(exp)). Division: DVE evacuates op to SBUF, reciprocal of
    row 64, PE broadcast (ones64^T @ recip), DVE multiply, DMA out.
"""

import sys

if "/opt/trn_rl_repo" not in sys.path:
    sys.path.insert(0, "/opt/trn_rl_repo")

import numpy as np

B, D, T, H = 4, 1024, 2048, 16
NHL = 8            # heads per core
DH = 64            # head dim
NT = T // 128      # 16 t-tiles
ND = D // 128      # 8 d-chunks
NST = T // 512     # 4 t-strips

_prog = None


def _build(loop_n=None, part=None):
    import contextlib
    import concourse.bass as bass
    import concourse.bacc as bacc
    import concourse.tile as tile
    from concourse import mybir
    from concourse.masks import make_identity

    f32 = mybir.dt.float32
    f32r = mybir.dt.float32r
    bf16 = mybir.dt.bfloat16
    AF = mybir.ActivationFunctionType
    ALU = mybir.AluOpType

    nc = bacc.Bacc()
    x_ext = nc.dram_tensor("x_local", [NT, 128, ND * 128], f32r, kind="ExternalInput")
    w_ext = nc.dram_tensor("w_local", [ND, 128, NHL * 192], f32r, kind="ExternalInput")
    out_ext = nc.dram_tensor("out_local", [NHL * DH, T], f32, kind="ExternalOutput")

    with tile.TileContext(nc) as tc:
      with (tc.For_i(0, loop_n, 1) if loop_n else contextlib.nullcontext()):
          with tc.tile_pool(name="persist", bufs=1) as persist, \
               tc.tile_pool(name="qkTp", bufs=1) as qkTp, \
               tc.tile_pool(name="vp", bufs=1) as vp:
              ident = persist.tile([128, 128], f32, tag="ident")
              make_identity(nc, ident)
              identb = persist.tile([128, 128], bf16, tag="identb")
              nc.vector.tensor_copy(identb, ident)
              ones64 = persist.tile([1, 64], f32r, tag="ones64")
              nc.vector.memset(ones64.bitcast(f32), 1.0)

              # qTd[h]: [128, T] qhat^T duplicated on partitions 0:64 and
              # 64:128; kTi[h]: [128, T/2] khat^T with even s-chunks on
              # partitions 0:64, odd on 64:128 (two K=64 score matmuls in
              # separate PE row-groups via tile_position).
              qTd = [qkTp.tile([128, T], bf16, tag=f"qTd{h}", name=f"qTd{h}")
                     for h in range(NHL)]
              kTi = [qkTp.tile([128, T // 2], bf16, tag=f"kTi{h}",
                               name=f"kTi{h}") for h in range(NHL)]
              # vt[:, sc, h, :]: [s 128, 65] = [v | 1] for s-chunk sc, head h
              vt = vp.tile([128, NT, NHL, 65], bf16, tag="vt", name="vt")
              nc.gpsimd.memset(vt[:, :, :, 64:65], 1.0)

              with tc.tile_pool(name="wsb", bufs=1) as wpool, \
                   tc.tile_pool(name="xin", bufs=2) as xpool, \
                   tc.tile_pool(name="stage", bufs=4) as stpool, \
                   tc.tile_pool(name="stats", bufs=4) as statp, \
                   tc.tile_pool(name="pt", bufs=4) as ptp, \
                   tc.tile_pool(name="dt", bufs=4) as dtp, \
                   tc.tile_pool(name="osb", bufs=3) as osbp, \
                   tc.tile_pool(name="outsb", bufs=4) as outp:
                  wsb = [wpool.tile([128, NHL * 192], f32r, tag=f"w{dc}",
                                    name=f"w{dc}") for dc in range(ND)]

                  # -------- Pass A: k/v projection for all 16 t-tiles
                  kstgs = {}
                  with tc.tile_pool(name="kvps", bufs=2, space="PSUM") as kvp, \
                       tc.tile_pool(name="trpA", bufs=2, space="PSUM") as trpa:
                    for ti in range(NT):
                      xb = xpool.tile([128, ND * 128], f32r, tag="x")
                      nc.sync.dma_start(out=xb, in_=x_ext[ti])
                      if ti == 0:
                          # w DMAs behind the first x tile so tile 0's
                          # matmuls aren't queued behind all of w
                          for dc in range(ND):
                              nc.sync.dma_start(out=wsb[dc], in_=w_ext[dc])
                      # [128t, (k|v, h, e)] (two banks)
                      ps = kvp.tile([128, 2, 8, 64], f32, tag="kv")
                      for dc in range(ND):
                          for n in range(2):
                              nc.tensor.matmul(
                                  ps[:, n],
                                  lhsT=xb[:, dc * 128:(dc + 1) * 128],
                                  rhs=wsb[dc][:, n * 512: n * 512 + 512],
                                  start=(dc == 0), stop=(dc == ND - 1))
                      st = statp.tile([128, 8, 6], f32, tag="stk")
                      mv = statp.tile([128, 8, 2], f32, tag="mvk")
                      for h in range(NHL):
                          nc.vector.bn_stats(st[:, h, :], ps[:, 0, h, :])
                          nc.vector.bn_aggr(mv[:, h, :], st[:, h, :])
                      # sigma = sqrt(var*64/63); inv = 1/(sigma+1e-5)
                      nc.scalar.activation(
                          mv[:, :, 1:2], mv[:, :, 1:2], AF.Sqrt,
                          scale=float(64.0 / 63.0))
                      nc.vector.tensor_scalar_add(
                          mv[:, :, 1:2], mv[:, :, 1:2], 1e-5)
                      nc.vector.reciprocal(mv[:, :, 1:2], mv[:, :, 1:2])
                      nm = statp.tile([128, 8, 1], f32, tag="nmk")
                      nc.vector.scalar_tensor_tensor(
                          out=nm, in0=mv[:, :, 0:1], scalar=-1.0,
                          in1=mv[:, :, 1:2], op0=ALU.mult, op1=ALU.mult)
                      # all 8 heads' v in one evacuation
                      nc.vector.tensor_copy(vt[:, ti, :, 0:64], ps[:, 1])
                      for h in range(NHL):
                          if ti % 2 == 0:
                              kstg = stpool.tile(
                                  [128, 128], bf16, tag=f"kstg{h}",
                                  name=f"kstg{h}", bufs=2)
                              kstgs[h] = kstg
                          else:
                              kstg = kstgs[h]
                          # khat = k*inv + (-mean*inv) on ACT (idle here)
                          nc.scalar.activation(
                              kstg[:, (ti % 2) * 64: (ti % 2) * 64 + 64],
                              ps[:, 0, h, :], AF.Identity,
                              bias=nm[:, h, :], scale=mv[:, h, 1:2])
                          if ti % 2 == 1:
                              trk = trpa.tile([128, 128], bf16,
                                              tag="tr", name="trk")
                              nc.tensor.transpose(trk, kstg, identb)
                              nc.vector.tensor_copy(
                                  kTi[h][:, (ti // 2) * 128:
                                         (ti // 2) * 128 + 128], trk)

                  # -------- Pass B: q projection fused with attention
                  with tc.tile_pool(name="qps", bufs=1, space="PSUM") as qpp, \
                       tc.tile_pool(name="trrep", bufs=1, space="PSUM") as trp, \
                       tc.tile_pool(name="spsum", bufs=2, space="PSUM") as spp, \
                       tc.tile_pool(name="opsum", bufs=1, space="PSUM") as opp:
                    qraws = {}
                    mv4s = {}

                    def emit_qproj(ti, slot, mv4):
                        # projection + stats; raw q parked in SBUF as bf16
                        xb = xpool.tile([128, ND * 128], f32r, tag="x")
                        nc.sync.dma_start(out=xb, in_=x_ext[ti])
                        ps = qpp.tile([128, 8, 64], f32, tag="q")
                        for dc in range(ND):
                            nc.tensor.matmul(
                                ps[:, :, :],
                                lhsT=xb[:, dc * 128:(dc + 1) * 128],
                                rhs=wsb[dc][:, 1024:1536],
                                start=(dc == 0), stop=(dc == ND - 1))
                        st = statp.tile([128, 8, 6], f32, tag="stq")
                        for h in range(NHL):
                            nc.vector.bn_stats(st[:, h, :], ps[:, h, :])
                            nc.vector.bn_aggr(mv4[:, slot, h, :], st[:, h, :])
                        qraw = stpool.tile([128, 8, 64], bf16, tag="qraw",
                                           name="qraw", bufs=6)
                        nc.vector.tensor_copy(qraw, ps)
                        qraws[ti] = qraw

                    def emit_qstats(mv4):
                        # one batched sigma->inv for 4 tiles x 8 heads
                        nc.scalar.activation(
                            mv4[:, :, :, 1:2], mv4[:, :, :, 1:2], AF.Sqrt,
                            scale=float(64.0 / 63.0))
                        nc.vector.tensor_scalar_add(
                            mv4[:, :, :, 1:2], mv4[:, :, :, 1:2], 1e-5)
                        nc.vector.reciprocal(
                            mv4[:, :, :, 1:2], mv4[:, :, :, 1:2])

                    def emit_qnorm(ti, slot, mv4):
                        qraw = qraws.pop(ti)
                        for h in range(NHL):
                            qstg = stpool.tile([128, 128], bf16,
                                               tag="qstg", name="qstg")
                            for u in range(2):
                                nc.vector.tensor_scalar(
                                    out=qstg[:, u * 64: u * 64 + 64],
                                    in0=qraw[:, h, :],
                                    scalar1=mv4[:, slot, h, 0:1],
                                    scalar2=mv4[:, slot, h, 1:2],
                                    op0=ALU.subtract, op1=ALU.mult)
                            trq = trp.tile([128, 128], bf16, tag="tr",
                                           name="trq")
                            nc.tensor.transpose(trq, qstg, identb)
                            nc.vector.tensor_copy(
                                qTd[h][:, ti * 128: (ti + 1) * 128], trq)

                    def emit_qtile_step(st_next, h):
                        # interleave next strip's q prep behind unit (h, st)
                        if st_next >= NST:
                            return
                        if h == 0:
                            mv4s[st_next] = statp.tile(
                                [128, 4, 8, 2], f32, tag="mv4", name="mv4",
                                bufs=2)
                        mv4 = mv4s[st_next]
                        if h < 4:
                            emit_qproj(st_next * 4 + h, h, mv4)
                            if h == 3:
                                emit_qstats(mv4)
                        else:
                            emit_qnorm(st_next * 4 + (h - 4), h - 4, mv4)

                    def emit_unit(h, st):
                        op = opp.tile([65, 512], f32, tag="op")
                        NJ = 8
                        LOOKAHEAD = 2
                        pts = []

                        def emit_scores(j):
                            sp = spp.tile([128, 1024], f32, tag="sp",
                                          name=f"sp{j}")
                            for u in range(2):
                                hb = u * 64
                                nc.tensor.matmul(
                                    sp[:, u * 512: (u + 1) * 512],
                                    lhsT=kTi[h][hb: hb + 64,
                                                j * 128: (j + 1) * 128],
                                    rhs=qTd[h][hb: hb + 64,
                                               st * 512: (st + 1) * 512],
                                    start=True, stop=True,
                                    tile_position=(hb, 0))
                            pt = ptp.tile([128, 1024], bf16, tag="pt",
                                          name=f"pt{j}")
                            nc.scalar.activation(pt, sp, AF.Exp, scale=0.125)
                            pts.append(pt)

                        def emit_pv(j):
                            for u in range(2):
                                sc = 2 * j + u
                                nc.tensor.matmul(
                                    op, lhsT=vt[:, sc, h, :],
                                    rhs=pts[j][:, u * 512: (u + 1) * 512],
                                    start=(sc == 0), stop=(sc == 15))

                        for j in range(LOOKAHEAD):
                            emit_scores(j)
                        for j in range(NJ):
                            if j + LOOKAHEAD < NJ:
                                emit_scores(j + LOOKAHEAD)
                            emit_pv(j)
                        osb = osbp.tile([65, 512], f32, tag="osb")
                        nc.vector.tensor_copy(osb, op)
                        rtf = dtp.tile([1, 512], f32, tag="rtf")
                        nc.vector.reciprocal(rtf, osb[64:65, :])
                        rtS = dtp.tile([1, 512], f32r, tag="rt")
                        nc.vector.tensor_copy(rtS, rtf)
                        rep = trp.tile([64, 512], f32, tag="rep")
                        nc.tensor.matmul(rep, lhsT=ones64, rhs=rtS,
                                         start=True, stop=True)
                        outt = outp.tile([64, 512], f32, tag="outt")
                        nc.vector.tensor_mul(outt, osb[0:64, :], rep)
                        nc.sync.dma_start(
                            out=out_ext[h * 64: (h + 1) * 64,
                                        st * 512: (st + 1) * 512],
                            in_=outt)

                    # prologue: strip 0's q tiles
                    for h in range(NHL):
                        emit_qtile_step(0, h)
                    for st in range(NST):
                        for h in range(NHL):
                            emit_unit(h, st)
                            emit_qtile_step(st + 1, h)
    nc.finalize()
    return nc


def _get_prog():
    global _prog
    if _prog is None:
        _prog = _build()
    return _prog


def make_in_maps(x, qkv):
    x = np.ascontiguousarray(np.asarray(x, dtype=np.float32))
    qkv = np.ascontiguousarray(np.asarray(qkv, dtype=np.float32))
    in_maps = []
    for c in range(8):
        b = c // 2
        hb = (c % 2) * 8
        # [16 ti, 128 dpart, 8 dc * 128 tf] (4KB contiguous per partition)
        xp = (x[b].reshape(ND, 128, NT, 128).transpose(2, 1, 0, 3)
              .reshape(NT, 128, ND * 128).copy())
        # w cols: [k h0..7 | v h0..7 | q h0..7] * 64
        a = qkv[:, hb:hb + 8]                 # [3(q,k,v), 8, D, 64]
        wp = np.empty((D, 24, DH), np.float32)
        wp[:, 0:8] = a[1].transpose(1, 0, 2)
        wp[:, 8:16] = a[2].transpose(1, 0, 2)
        wp[:, 16:24] = a[0].transpose(1, 0, 2)
        wp = wp.reshape(D, NHL * 192).reshape(ND, 128, NHL * 192).copy()
        in_maps.append({"x_local": xp, "w_local": wp})
    return in_maps


def gather(results):
    out = np.empty((B, D, T), np.float32)
    for c in range(8):
        out[c // 2, (c % 2) * 512: (c % 2) * 512 + 512, :] = \
            results[c]["out_local"]
    return out


def kernel(**inputs):
    from concourse.bass_utils import run_bass_kernel_spmd

    nc = _get_prog()
    in_maps = make_in_maps(inputs["x"], inputs["qkv"])
    res = run_bass_kernel_spmd(nc, in_maps, list(range(8)))
    return gather(res.results)


# revision 22
# speedup vs baseline: 1.3256x; 1.3256x over previous
"""Trainium2 Bass kernel for fused MHA with q/k std-normalization.

Reference computation (per batch b, head h):
    q,k,v = x[b].T @ Wq/Wk/Wv          [T, 64] each
    q = (q - mean_e) / (std_e(ddof=1) + 1e-5)   (same for k)
    attn = softmax(q @ k.T / 8)
    out[b, h*64:(h+1)*64, :] = (attn @ v).T

Sharding: 8 cores = 4 batches x 2 half-head-groups. Core c handles batch
c//2, heads (c%2)*8 .. (c%2)*8+8. Fully head-independent, no collectives.

Per-core pipeline, structured so the ACT-bound softmax exp (the largest
irreducible per-engine cost, ~266us busy) overlaps the q-projection and
the attention PE work instead of idling during a separate QKV phase:

  Pass A (k/v projection, 16 t-tiles): per tile one psum tile
    [128t, (k|v, h, e)] accumulated over 8 d-chunks (two N=512 f32r
    matmuls per chunk). Per-head bn_stats/bn_aggr on the k block read
    PSUM directly; one batched ACT sqrt per tile covers 8 sigmas; DVE
    computes eps+reciprocal and -mean*inv; khat = k*inv + (-mean*inv)
    runs on the otherwise-idle ACT (Identity with per-partition
    scale/bias APs) into bf16 kstg, PE-transposed (1 cyc/row) into kTi;
    all 8 heads' v evacuated bf16 with a single DVE copy into vt
    ([s 128, 64 v | 1]; the ones column feeds the softmax denominator
    through the attention*V matmul).

  Pass B (q projection fused with attention): the next strip's four
    q-tiles are prepared interleaved between attention units: units
    h=0..3 are followed by one q-tile projection each (per-head
    bn_stats from PSUM, raw q parked bf16 in SBUF to free the psum
    bank), after h=3 ONE batched ACT sqrt covers all 4 tiles x 8 heads
    (2 act-table switches per strip total), and units h=4..7 are
    followed by one tile's normalize (DVE 2-byte fast mode from bf16
    raw q, duplicated into both column halves) + PE transpose +
    evacuation into qTd.

    Attention unit (h, st): scores^T [s,t] = kT-chunk.T @ qT-strip
    (K=64 bf16, chunk pairs in separate PE row-groups via
    tile_position); exp on ACT with scale=1/8 folded in (|q||k|/8 <= 8
    so max-subtraction-free softmax is fp32-safe) producing bf16 probs;
    attention*V (bf16) accumulates op [65, 512] over 16 s-chunks (row
    64 = sum(exp)). Division: DVE evacuates op to SBUF, reciprocal of
    row 64, PE broadcast (ones64^T @ recip), DVE multiply, DMA out.
"""

import sys

if "/opt/trn_rl_repo" not in sys.path:
    sys.path.insert(0, "/opt/trn_rl_repo")

import numpy as np

B, D, T, H = 4, 1024, 2048, 16
NHL = 8            # heads per core
DH = 64            # head dim
NT = T // 128      # 16 t-tiles
ND = D // 128      # 8 d-chunks
NST = T // 512     # 4 t-strips

_prog = None


def _build(loop_n=None, part=None):
    import contextlib
    import concourse.bass as bass
    import concourse.bacc as bacc
    import concourse.tile as tile
    from concourse import mybir
    from concourse.masks import make_identity

    f32 = mybir.dt.float32
    f32r = mybir.dt.float32r
    bf16 = mybir.dt.bfloat16
    AF = mybir.ActivationFunctionType
    ALU = mybir.AluOpType

    nc = bacc.Bacc()
    x_ext = nc.dram_tensor("x_local", [NT, 128, ND * 128], f32r, kind="ExternalInput")
    w_ext = nc.dram_tensor("w_local", [ND, 128, NHL * 192], f32r, kind="ExternalInput")
    out_ext = nc.dram_tensor("out_local", [NHL * DH, T], f32, kind="ExternalOutput")

    with tile.TileContext(nc) as tc:
      with (tc.For_i(0, loop_n, 1) if loop_n else contextlib.nullcontext()):
          with tc.tile_pool(name="persist", bufs=1) as persist, \
               tc.tile_pool(name="qkTp", bufs=1) as qkTp, \
               tc.tile_pool(name="vp", bufs=1) as vp:
              ident = persist.tile([128, 128], f32, tag="ident")
              make_identity(nc, ident)
              identb = persist.tile([128, 128], bf16, tag="identb")
              nc.vector.tensor_copy(identb, ident)
              ones64 = persist.tile([1, 64], f32r, tag="ones64")
              nc.vector.memset(ones64.bitcast(f32), 1.0)

              # qTd[h]: [128, T] qhat^T duplicated on partitions 0:64 and
              # 64:128; kTi[h]: [128, T/2] khat^T with even s-chunks on
              # partitions 0:64, odd on 64:128 (two K=64 score matmuls in
              # separate PE row-groups via tile_position).
              qTd = [qkTp.tile([128, T], bf16, tag=f"qTd{h}", name=f"qTd{h}")
                     for h in range(NHL)]
              kTi = [qkTp.tile([128, T // 2], bf16, tag=f"kTi{h}",
                               name=f"kTi{h}") for h in range(NHL)]
              # vt[:, sc, h, :]: [s 128, 65] = [v | 1] for s-chunk sc, head h
              vt = vp.tile([128, NT, NHL, 65], bf16, tag="vt", name="vt")
              nc.gpsimd.memset(vt[:, :, :, 64:65], 1.0)

              with tc.tile_pool(name="wsb", bufs=1) as wpool, \
                   tc.tile_pool(name="xin", bufs=2) as xpool, \
                   tc.tile_pool(name="stage", bufs=4) as stpool, \
                   tc.tile_pool(name="stats", bufs=4) as statp, \
                   tc.tile_pool(name="pt", bufs=4) as ptp, \
                   tc.tile_pool(name="dt", bufs=4) as dtp, \
                   tc.tile_pool(name="osb", bufs=3) as osbp, \
                   tc.tile_pool(name="outsb", bufs=4) as outp:
                  wsb = [wpool.tile([128, NHL * 192], f32r, tag=f"w{dc}",
                                    name=f"w{dc}") for dc in range(ND)]

                  # -------- Pass A: k/v projection for all 16 t-tiles
                  kstgs = {}
                  pend_trk = []
                  with tc.tile_pool(name="kvps", bufs=2, space="PSUM") as kvp, \
                       tc.tile_pool(name="trpA", bufs=2, space="PSUM") as trpa:
                    def drain_trk(n):
                        for _ in range(min(n, len(pend_trk))):
                            kstg, h, tio = pend_trk.pop(0)
                            trk = trpa.tile([128, 128], bf16,
                                            tag="tr", name="trk")
                            nc.tensor.transpose(trk, kstg, identb)
                            nc.vector.tensor_copy(
                                kTi[h][:, (tio // 2) * 128:
                                       (tio // 2) * 128 + 128], trk)

                    for ti in range(NT if part != "pB" else 0):
                      xb = xpool.tile([128, ND * 128], f32r, tag="x")
                      nc.sync.dma_start(out=xb, in_=x_ext[ti])
                      if ti == 0:
                          # w DMAs behind the first x tile so tile 0's
                          # matmuls aren't queued behind all of w
                          for dc in range(ND):
                              nc.sync.dma_start(out=wsb[dc], in_=w_ext[dc])
                      # [128t, (k|v, h, e)] (two banks)
                      ps = kvp.tile([128, 2, 8, 64], f32, tag="kv")
                      for dc in range(ND):
                          for n in range(2):
                              nc.tensor.matmul(
                                  ps[:, n],
                                  lhsT=xb[:, dc * 128:(dc + 1) * 128],
                                  rhs=wsb[dc][:, n * 512: n * 512 + 512],
                                  start=(dc == 0), stop=(dc == ND - 1))
                      # previous pair's transposes: all inputs ready, so
                      # they stream without head-of-line blocking the PE
                      drain_trk(99)
                      st = statp.tile([128, 8, 6], f32, tag="stk")
                      mv = statp.tile([128, 8, 2], f32, tag="mvk")
                      for h in range(NHL):
                          nc.vector.bn_stats(st[:, h, :], ps[:, 0, h, :])
                          nc.vector.bn_aggr(mv[:, h, :], st[:, h, :])
                      # sigma = sqrt(var*64/63); inv = 1/(sigma+1e-5)
                      nc.scalar.activation(
                          mv[:, :, 1:2], mv[:, :, 1:2], AF.Sqrt,
                          scale=float(64.0 / 63.0))
                      nc.vector.tensor_scalar_add(
                          mv[:, :, 1:2], mv[:, :, 1:2], 1e-5)
                      nc.vector.reciprocal(mv[:, :, 1:2], mv[:, :, 1:2])
                      nm = statp.tile([128, 8, 1], f32, tag="nmk")
                      nc.vector.scalar_tensor_tensor(
                          out=nm, in0=mv[:, :, 0:1], scalar=-1.0,
                          in1=mv[:, :, 1:2], op0=ALU.mult, op1=ALU.mult)
                      # all 8 heads' v in one evacuation
                      nc.vector.tensor_copy(vt[:, ti, :, 0:64], ps[:, 1])
                      for h in range(NHL):
                          if ti % 2 == 0:
                              kstg = stpool.tile(
                                  [128, 128], bf16, tag=f"kstg{h}",
                                  name=f"kstg{h}", bufs=2)
                              kstgs[h] = kstg
                          else:
                              kstg = kstgs[h]
                          # khat = k*inv + (-mean*inv) on ACT (idle here)
                          nc.scalar.activation(
                              kstg[:, (ti % 2) * 64: (ti % 2) * 64 + 64],
                              ps[:, 0, h, :], AF.Identity,
                              bias=nm[:, h, :], scale=mv[:, h, 1:2])
                          if ti % 2 == 1:
                              pend_trk.append((kstg, h, ti))

                    drain_trk(99)

                  # -------- Pass B: q projection fused with attention
                  with tc.tile_pool(name="qps", bufs=1, space="PSUM") as qpp, \
                       tc.tile_pool(name="trrep", bufs=1, space="PSUM") as trp, \
                       tc.tile_pool(name="spsum", bufs=2, space="PSUM") as spp, \
                       tc.tile_pool(name="opsum", bufs=1, space="PSUM") as opp:
                    qraws = {}
                    mv4s = {}

                    def emit_qproj(ti, slot, mv4):
                        # projection + stats; raw q parked in SBUF as bf16
                        xb = xpool.tile([128, ND * 128], f32r, tag="x")
                        nc.sync.dma_start(out=xb, in_=x_ext[ti])
                        ps = qpp.tile([128, 8, 64], f32, tag="q")
                        for dc in range(ND):
                            nc.tensor.matmul(
                                ps[:, :, :],
                                lhsT=xb[:, dc * 128:(dc + 1) * 128],
                                rhs=wsb[dc][:, 1024:1536],
                                start=(dc == 0), stop=(dc == ND - 1))
                        st = statp.tile([128, 8, 6], f32, tag="stq")
                        for h in range(NHL):
                            nc.vector.bn_stats(st[:, h, :], ps[:, h, :])
                            nc.vector.bn_aggr(mv4[:, slot, h, :], st[:, h, :])
                        qraw = stpool.tile([128, 8, 64], bf16, tag="qraw",
                                           name="qraw", bufs=6)
                        nc.vector.tensor_copy(qraw, ps)
                        qraws[ti] = qraw

                    def emit_qstats(mv4):
                        # one batched sigma->inv for 4 tiles x 8 heads
                        nc.scalar.activation(
                            mv4[:, :, :, 1:2], mv4[:, :, :, 1:2], AF.Sqrt,
                            scale=float(64.0 / 63.0))
                        nc.vector.tensor_scalar_add(
                            mv4[:, :, :, 1:2], mv4[:, :, :, 1:2], 1e-5)
                        nc.vector.reciprocal(
                            mv4[:, :, :, 1:2], mv4[:, :, :, 1:2])

                    pending_tr = []

                    def emit_qnorm(ti, slot, mv4):
                        # normalize now (DVE); queue the transpose+evac to be
                        # spread across the next unit's j-steps so the PE
                        # doesn't head-of-line block on DVE evacuations
                        qraw = qraws.pop(ti)
                        for h in range(NHL):
                            qstg = stpool.tile([128, 128], bf16,
                                               tag="qstg", name="qstg",
                                               bufs=18)
                            for u in range(2):
                                nc.vector.tensor_scalar(
                                    out=qstg[:, u * 64: u * 64 + 64],
                                    in0=qraw[:, h, :],
                                    scalar1=mv4[:, slot, h, 0:1],
                                    scalar2=mv4[:, slot, h, 1:2],
                                    op0=ALU.subtract, op1=ALU.mult)
                            pending_tr.append((qstg, h, ti))

                    def drain_tr(n):
                        for _ in range(min(n, len(pending_tr))):
                            qstg, h, ti = pending_tr.pop(0)
                            trq = trp.tile([128, 128], bf16, tag="tr",
                                           name="trq")
                            nc.tensor.transpose(trq, qstg, identb)
                            nc.vector.tensor_copy(
                                qTd[h][:, ti * 128: (ti + 1) * 128], trq)

                    def emit_qtile_step(st_next, h):
                        # interleave next strip's q prep behind unit (h, st)
                        if st_next >= NST:
                            return
                        if h == 0:
                            mv4s[st_next] = statp.tile(
                                [128, 4, 8, 2], f32, tag="mv4", name="mv4",
                                bufs=2)
                        mv4 = mv4s[st_next]
                        if h < 4:
                            emit_qproj(st_next * 4 + h, h, mv4)
                            if h == 3:
                                emit_qstats(mv4)
                        else:
                            emit_qnorm(st_next * 4 + (h - 4), h - 4, mv4)

                    def emit_unit(h, st):
                        op = opp.tile([65, 512], f32, tag="op")
                        NJ = 8
                        LOOKAHEAD = 2
                        pts = []

                        def emit_scores(j):
                            sp = spp.tile([128, 1024], f32, tag="sp",
                                          name=f"sp{j}")
                            for u in range(2):
                                hb = u * 64
                                nc.tensor.matmul(
                                    sp[:, u * 512: (u + 1) * 512],
                                    lhsT=kTi[h][hb: hb + 64,
                                                j * 128: (j + 1) * 128],
                                    rhs=qTd[h][hb: hb + 64,
                                               st * 512: (st + 1) * 512],
                                    start=True, stop=True,
                                    tile_position=(hb, 0))
                            pt = ptp.tile([128, 1024], bf16, tag="pt",
                                          name=f"pt{j}")
                            nc.scalar.activation(pt, sp, AF.Exp, scale=0.125)
                            pts.append(pt)

                        def emit_pv(j):
                            for u in range(2):
                                sc = 2 * j + u
                                nc.tensor.matmul(
                                    op, lhsT=vt[:, sc, h, :],
                                    rhs=pts[j][:, u * 512: (u + 1) * 512],
                                    start=(sc == 0), stop=(sc == 15))

                        for j in range(LOOKAHEAD):
                            emit_scores(j)
                        for j in range(NJ):
                            if j + LOOKAHEAD < NJ:
                                emit_scores(j + LOOKAHEAD)
                            emit_pv(j)
                            drain_tr(1)
                        osb = osbp.tile([65, 512], f32, tag="osb")
                        nc.vector.tensor_copy(osb, op)
                        rtf = dtp.tile([1, 512], f32, tag="rtf")
                        nc.vector.reciprocal(rtf, osb[64:65, :])
                        rtS = dtp.tile([1, 512], f32r, tag="rt")
                        nc.vector.tensor_copy(rtS, rtf)
                        rep = trp.tile([64, 512], f32, tag="rep")
                        nc.tensor.matmul(rep, lhsT=ones64, rhs=rtS,
                                         start=True, stop=True)
                        outt = outp.tile([64, 512], f32, tag="outt")
                        nc.vector.tensor_mul(outt, osb[0:64, :], rep)
                        nc.sync.dma_start(
                            out=out_ext[h * 64: (h + 1) * 64,
                                        st * 512: (st + 1) * 512],
                            in_=outt)

                    # prologue: strip 0's q tiles
                    if part != "pA":
                        for h in range(NHL):
                            emit_qtile_step(0, h)
                        for st in range(NST):
                            for h in range(NHL):
                                if h == 0:
                                    drain_tr(99)  # flush before consumers
                                if part != "qonly":
                                    emit_unit(h, st)
                                emit_qtile_step(st + 1, h)
                        drain_tr(99)
    nc.finalize()
    return nc


def _get_prog():
    global _prog
    if _prog is None:
        _prog = _build()
    return _prog


def make_in_maps(x, qkv):
    x = np.ascontiguousarray(np.asarray(x, dtype=np.float32))
    qkv = np.ascontiguousarray(np.asarray(qkv, dtype=np.float32))
    in_maps = []
    for c in range(8):
        b = c // 2
        hb = (c % 2) * 8
        # [16 ti, 128 dpart, 8 dc * 128 tf] (4KB contiguous per partition)
        xp = (x[b].reshape(ND, 128, NT, 128).transpose(2, 1, 0, 3)
              .reshape(NT, 128, ND * 128).copy())
        # w cols: [k h0..7 | v h0..7 | q h0..7] * 64
        a = qkv[:, hb:hb + 8]                 # [3(q,k,v), 8, D, 64]
        wp = np.empty((D, 24, DH), np.float32)
        wp[:, 0:8] = a[1].transpose(1, 0, 2)
        wp[:, 8:16] = a[2].transpose(1, 0, 2)
        wp[:, 16:24] = a[0].transpose(1, 0, 2)
        wp = wp.reshape(D, NHL * 192).reshape(ND, 128, NHL * 192).copy()
        in_maps.append({"x_local": xp, "w_local": wp})
    return in_maps


def gather(results):
    out = np.empty((B, D, T), np.float32)
    for c in range(8):
        out[c // 2, (c % 2) * 512: (c % 2) * 512 + 512, :] = \
            results[c]["out_local"]
    return out


def kernel(**inputs):
    from concourse.bass_utils import run_bass_kernel_spmd

    nc = _get_prog()
    in_maps = make_in_maps(inputs["x"], inputs["qkv"])
    res = run_bass_kernel_spmd(nc, in_maps, list(range(8)))
    return gather(res.results)


# revision 31
# speedup vs baseline: 1.7504x; 1.3205x over previous
"""Trainium2 Bass kernel for fused MHA with q/k std-normalization.

Reference computation (per batch b, head h):
    q,k,v = x[b].T @ Wq/Wk/Wv          [T, 64] each
    q = (q - mean_e) / (std_e(ddof=1) + 1e-5)   (same for k)
    attn = softmax(q @ k.T / 8)
    out[b, h*64:(h+1)*64, :] = (attn @ v).T

Sharding: 8 cores = 4 batches x 2 half-head-groups. Core c handles batch
c//2, heads (c%2)*8 .. (c%2)*8+8. Fully head-independent, no collectives.

Per-core pipeline, structured so the ACT-bound softmax exp (the largest
irreducible per-engine cost, ~266us busy) overlaps the q-projection and
the attention PE work instead of idling during a separate QKV phase:

  Pass A (k/v projection, 16 t-tiles): per tile one psum tile
    [128t, (k|v, h, e)] accumulated over 8 d-chunks (two N=512 f32r
    matmuls per chunk). Per-head bn_stats/bn_aggr on the k block read
    PSUM directly; one batched ACT sqrt per tile covers 8 sigmas; DVE
    computes eps+reciprocal and -mean*inv; khat = k*inv + (-mean*inv)
    runs on the otherwise-idle ACT (Identity with per-partition
    scale/bias APs) into bf16 kstg, PE-transposed (1 cyc/row) into kTi;
    all 8 heads' v evacuated bf16 with a single DVE copy into vt
    ([s 128, 64 v | 1]; the ones column feeds the softmax denominator
    through the attention*V matmul).

  Pass B (q projection fused with attention): the next strip's four
    q-tiles are prepared interleaved between attention units: units
    h=0..3 are followed by one q-tile projection each (per-head
    bn_stats from PSUM, raw q parked bf16 in SBUF to free the psum
    bank), after h=3 ONE batched ACT sqrt covers all 4 tiles x 8 heads
    (2 act-table switches per strip total), and units h=4..7 are
    followed by one tile's normalize (DVE 2-byte fast mode from bf16
    raw q, duplicated into both column halves) + PE transpose +
    evacuation into qTd.

    Attention unit (h, st): scores^T [s,t] = kT-chunk.T @ qT-strip
    (K=64 bf16, chunk pairs in separate PE row-groups via
    tile_position); exp on ACT with scale=1/8 folded in (|q||k|/8 <= 8
    so max-subtraction-free softmax is fp32-safe) producing bf16 probs;
    attention*V (bf16) accumulates op [65, 512] over 16 s-chunks (row
    64 = sum(exp)). Division: DVE evacuates op to SBUF, reciprocal of
    row 64, PE broadcast (ones64^T @ recip), DVE multiply, DMA out.
"""

import sys

if "/opt/trn_rl_repo" not in sys.path:
    sys.path.insert(0, "/opt/trn_rl_repo")

import numpy as np

B, D, T, H = 4, 1024, 2048, 16
NHL = 8            # heads per core
DH = 64            # head dim
NT = T // 128      # 16 t-tiles
ND = D // 128      # 8 d-chunks
NST = T // 512     # 4 t-strips

_prog = None


def _build(loop_n=None, part=None):
    import contextlib
    import concourse.bass as bass
    import concourse.bacc as bacc
    import concourse.tile as tile
    from concourse import mybir
    from concourse.masks import make_identity

    f32 = mybir.dt.float32
    f32r = mybir.dt.float32r
    bf16 = mybir.dt.bfloat16
    AF = mybir.ActivationFunctionType
    ALU = mybir.AluOpType

    nc = bacc.Bacc()
    x_ext = nc.dram_tensor("x_local", [NT, 128, ND * 128], f32r, kind="ExternalInput")
    w_ext = nc.dram_tensor("w_local", [ND, 128, NHL * 192], f32r, kind="ExternalInput")
    out_ext = nc.dram_tensor("out_local", [NHL * DH, T], f32, kind="ExternalOutput")

    with tile.TileContext(nc) as tc:
      with (tc.For_i(0, loop_n, 1) if loop_n else contextlib.nullcontext()):
          with tc.tile_pool(name="persist", bufs=1) as persist, \
               tc.tile_pool(name="qkTp", bufs=1) as qkTp, \
               tc.tile_pool(name="vp", bufs=1) as vp:
              ident = persist.tile([128, 128], f32, tag="ident")
              make_identity(nc, ident)
              identb = persist.tile([128, 128], bf16, tag="identb")
              nc.vector.tensor_copy(identb, ident)
              ones64 = persist.tile([1, 64], f32r, tag="ones64")
              nc.vector.memset(ones64.bitcast(f32), 1.0)

              # qTd[h]: [128, T] qhat^T duplicated on partitions 0:64 and
              # 64:128; kTi[h]: [128, T/2] khat^T with even s-chunks on
              # partitions 0:64, odd on 64:128 (two K=64 score matmuls in
              # separate PE row-groups via tile_position).
              qTd = [qkTp.tile([128, T], bf16, tag=f"qTd{h}", name=f"qTd{h}")
                     for h in range(NHL)]
              kTi = [qkTp.tile([128, T // 2], bf16, tag=f"kTi{h}",
                               name=f"kTi{h}") for h in range(NHL)]
              # vt[:, sc, h, :]: [s 128, 65] = [v | 1] for s-chunk sc, head h
              vt = vp.tile([128, NT, NHL, 65], bf16, tag="vt", name="vt")
              nc.gpsimd.memset(vt[:, :, :, 64:65], 1.0)

              with tc.tile_pool(name="wsb", bufs=1) as wpool, \
                   tc.tile_pool(name="xin", bufs=2) as xpool, \
                   tc.tile_pool(name="stage", bufs=4) as stpool, \
                   tc.tile_pool(name="stats", bufs=4) as statp, \
                   tc.tile_pool(name="pt", bufs=4) as ptp, \
                   tc.tile_pool(name="dt", bufs=4) as dtp, \
                   tc.tile_pool(name="osb", bufs=3) as osbp, \
                   tc.tile_pool(name="outsb", bufs=4) as outp:
                  wsb = [wpool.tile([128, NHL * 192], f32r, tag=f"w{dc}",
                                    name=f"w{dc}") for dc in range(ND)]

                  # -------- Pass A: k/v projection for all 16 t-tiles
                  kstgs = {}
                  pend_trk = []
                  with tc.tile_pool(name="kvps", bufs=3, space="PSUM") as kvp, \
                       tc.tile_pool(name="trpA", bufs=2, space="PSUM") as trpa:
                    def drain_trk_now(kstg, h, tio):
                        trk = trpa.tile([128, 128], bf16,
                                        tag="tr", name="trk")
                        nc.tensor.transpose(trk, kstg, identb)
                        nc.vector.tensor_copy(
                            kTi[h][:, (tio // 2) * 128:
                                   (tio // 2) * 128 + 128], trk)

                    for ti in range(NT if part != "pB" else 0):
                      xb = xpool.tile([128, ND * 128], f32r, tag="x")
                      nc.sync.dma_start(out=xb, in_=x_ext[ti])
                      if ti == 0:
                          # w DMAs behind the first x tile so tile 0's
                          # matmuls aren't queued behind all of w
                          for dc in range(ND):
                              nc.sync.dma_start(out=wsb[dc], in_=w_ext[dc])
                      # [128t, (k|v, h, e)] (two banks)
                      ps = kvp.tile([128, 2, 8, 64], f32, tag="kv")
                      for dc in range(ND):
                          for n in range(2):
                              nc.tensor.matmul(
                                  ps[:, n],
                                  lhsT=xb[:, dc * 128:(dc + 1) * 128],
                                  rhs=wsb[dc][:, n * 512: n * 512 + 512],
                                  start=(dc == 0), stop=(dc == ND - 1))

                      st = statp.tile([128, 8, 6], f32, tag="stk")
                      mv = statp.tile([128, 8, 2], f32, tag="mvk")
                      for h in range(NHL):
                          nc.vector.bn_stats(st[:, h, :], ps[:, 0, h, :])
                          nc.vector.bn_aggr(mv[:, h, :], st[:, h, :])
                      # sigma = sqrt(var*64/63); inv = 1/(sigma+1e-5)
                      nc.scalar.activation(
                          mv[:, :, 1:2], mv[:, :, 1:2], AF.Sqrt,
                          scale=float(64.0 / 63.0))
                      nc.vector.tensor_scalar_add(
                          mv[:, :, 1:2], mv[:, :, 1:2], 1e-5)
                      nc.vector.reciprocal(mv[:, :, 1:2], mv[:, :, 1:2])
                      nm = statp.tile([128, 8, 1], f32, tag="nmk")
                      nc.vector.scalar_tensor_tensor(
                          out=nm, in0=mv[:, :, 0:1], scalar=-1.0,
                          in1=mv[:, :, 1:2], op0=ALU.mult, op1=ALU.mult)
                      # all 8 heads' v in one evacuation
                      nc.vector.tensor_copy(vt[:, ti, :, 0:64], ps[:, 1])
                      for h in range(NHL):
                          if ti % 2 == 0:
                              kstg = stpool.tile(
                                  [128, 128], bf16, tag=f"kstg{h}",
                                  name=f"kstg{h}", bufs=2)
                              kstgs[h] = kstg
                          else:
                              kstg = kstgs[h]
                          # khat = k*inv + (-mean*inv) on ACT (idle here)
                          nc.scalar.activation(
                              kstg[:, (ti % 2) * 64: (ti % 2) * 64 + 64],
                              ps[:, 0, h, :], AF.Identity,
                              bias=nm[:, h, :], scale=mv[:, h, 1:2])
                          if ti % 2 == 1:
                              drain_trk_now(kstg, h, ti)

                  # -------- Pass B: q projection fused with attention
                  with tc.tile_pool(name="qps", bufs=1, space="PSUM") as qpp, \
                       tc.tile_pool(name="trrep", bufs=1, space="PSUM") as trp, \
                       tc.tile_pool(name="spsum", bufs=2, space="PSUM") as spp, \
                       tc.tile_pool(name="opsum", bufs=1, space="PSUM") as opp:
                    qraws = {}
                    mv4s = {}

                    def emit_qproj(ti, slot, mv4):
                        # projection + stats; raw q parked in SBUF as bf16
                        xb = xpool.tile([128, ND * 128], f32r, tag="x")
                        nc.sync.dma_start(out=xb, in_=x_ext[ti])
                        ps = qpp.tile([128, 8, 64], f32, tag="q")
                        for dc in range(ND):
                            nc.tensor.matmul(
                                ps[:, :, :],
                                lhsT=xb[:, dc * 128:(dc + 1) * 128],
                                rhs=wsb[dc][:, 1024:1536],
                                start=(dc == 0), stop=(dc == ND - 1))
                        st = statp.tile([128, 8, 6], f32, tag="stq")
                        for h in range(NHL):
                            nc.vector.bn_stats(st[:, h, :], ps[:, h, :])
                            nc.vector.bn_aggr(mv4[:, slot, h, :], st[:, h, :])
                        qraw = stpool.tile([128, 8, 64], bf16, tag="qraw",
                                           name="qraw", bufs=6)
                        nc.vector.tensor_copy(qraw, ps)
                        qraws[ti] = qraw

                    def emit_qstats(mv4):
                        # one batched sigma->inv for 4 tiles x 8 heads
                        nc.scalar.activation(
                            mv4[:, :, :, 1:2], mv4[:, :, :, 1:2], AF.Sqrt,
                            scale=float(64.0 / 63.0))
                        nc.vector.tensor_scalar_add(
                            mv4[:, :, :, 1:2], mv4[:, :, :, 1:2], 1e-5)
                        nc.vector.reciprocal(
                            mv4[:, :, :, 1:2], mv4[:, :, :, 1:2])

                    pending_tr = []

                    def emit_qnorm(ti, slot, mv4):
                        # normalize now (DVE); queue the transpose+evac to be
                        # spread across the next unit's j-steps so the PE
                        # doesn't head-of-line block on DVE evacuations
                        qraw = qraws.pop(ti)
                        for h in range(NHL):
                            qstg = stpool.tile([128, 128], bf16,
                                               tag="qstg", name="qstg",
                                               bufs=18)
                            for u in range(2):
                                nc.vector.tensor_scalar(
                                    out=qstg[:, u * 64: u * 64 + 64],
                                    in0=qraw[:, h, :],
                                    scalar1=mv4[:, slot, h, 0:1],
                                    scalar2=mv4[:, slot, h, 1:2],
                                    op0=ALU.subtract, op1=ALU.mult)
                            pending_tr.append((qstg, h, ti))

                    def drain_tr(n):
                        for _ in range(min(n, len(pending_tr))):
                            qstg, h, ti = pending_tr.pop(0)
                            trq = trp.tile([128, 128], bf16, tag="tr",
                                           name="trq")
                            nc.tensor.transpose(trq, qstg, identb)
                            nc.vector.tensor_copy(
                                qTd[h][:, ti * 128: (ti + 1) * 128], trq)

                    def emit_qtile_step(st_next, h):
                        # interleave next strip's q prep behind unit (h, st)
                        if st_next >= NST:
                            return
                        if h == 0:
                            mv4s[st_next] = statp.tile(
                                [128, 4, 8, 2], f32, tag="mv4", name="mv4",
                                bufs=2)
                        mv4 = mv4s[st_next]
                        if h < 4:
                            emit_qproj(st_next * 4 + h, h, mv4)
                            if h == 3:
                                emit_qstats(mv4)
                        elif h < 6:
                            emit_qnorm(st_next * 4 + (h - 4), h - 4, mv4)
                        elif h == 6:
                            emit_qnorm(st_next * 4 + 2, 2, mv4)
                            emit_qnorm(st_next * 4 + 3, 3, mv4)

                    def emit_unit(h, st):
                        op = opp.tile([65, 512], f32, tag="op")
                        NJ = 8
                        LOOKAHEAD = 2
                        pts = []

                        def emit_scores(j):
                            sp = spp.tile([128, 1024], f32, tag="sp",
                                          name=f"sp{j}")
                            for u in range(2):
                                hb = u * 64
                                nc.tensor.matmul(
                                    sp[:, u * 512: (u + 1) * 512],
                                    lhsT=kTi[h][hb: hb + 64,
                                                j * 128: (j + 1) * 128],
                                    rhs=qTd[h][hb: hb + 64,
                                               st * 512: (st + 1) * 512],
                                    start=True, stop=True,
                                    tile_position=(hb, 0))
                            pt = ptp.tile([128, 1024], bf16, tag="pt",
                                          name=f"pt{j}")
                            nc.scalar.activation(pt, sp, AF.Exp, scale=0.125)
                            pts.append(pt)

                        def emit_pv(j):
                            for u in range(2):
                                sc = 2 * j + u
                                nc.tensor.matmul(
                                    op, lhsT=vt[:, sc, h, :],
                                    rhs=pts[j][:, u * 512: (u + 1) * 512],
                                    start=(sc == 0), stop=(sc == 15))

                        for j in range(LOOKAHEAD):
                            emit_scores(j)
                        for j in range(NJ):
                            if j + LOOKAHEAD < NJ:
                                emit_scores(j + LOOKAHEAD)
                            emit_pv(j)
                            drain_tr(2)

                        def emit_div():
                            osb = osbp.tile([65, 512], f32, tag="osb")
                            nc.vector.tensor_copy(osb, op)
                            rtf = dtp.tile([1, 512], f32, tag="rtf")
                            nc.vector.reciprocal(rtf, osb[64:65, :])
                            rtS = dtp.tile([1, 512], f32r, tag="rt")
                            nc.vector.tensor_copy(rtS, rtf)
                            rep = trp.tile([64, 512], f32, tag="rep")
                            nc.tensor.matmul(rep, lhsT=ones64, rhs=rtS,
                                             start=True, stop=True)
                            outt = outp.tile([64, 512], f32, tag="outt")
                            nc.vector.tensor_mul(outt, osb[0:64, :], rep)
                            nc.sync.dma_start(
                                out=out_ext[h * 64: (h + 1) * 64,
                                            st * 512: (st + 1) * 512],
                                in_=outt)
                        return emit_div

                    # prologue: strip 0's q tiles
                    if part != "pA":
                        for h in range(NHL):
                            emit_qtile_step(0, h)
                        for st in range(NST):
                            for h in range(NHL):
                                if h == 0:
                                    drain_tr(99)  # flush before consumers
                                div = (emit_unit(h, st)
                                       if part != "qonly" else None)
                                if h == 3:
                                    # q stats land on DVE before the division
                                    # chain so the strip-boundary ACT sqrt
                                    # isn't blocked behind it
                                    emit_qtile_step(st + 1, h)
                                    if div is not None:
                                        div()
                                else:
                                    if div is not None:
                                        div()
                                    emit_qtile_step(st + 1, h)
                        drain_tr(99)
    nc.finalize()
    return nc


def _get_prog():
    global _prog
    if _prog is None:
        _prog = _build()
    return _prog


def make_in_maps(x, qkv):
    x = np.ascontiguousarray(np.asarray(x, dtype=np.float32))
    qkv = np.ascontiguousarray(np.asarray(qkv, dtype=np.float32))
    in_maps = []
    for c in range(8):
        b = c // 2
        hb = (c % 2) * 8
        # [16 ti, 128 dpart, 8 dc * 128 tf] (4KB contiguous per partition)
        xp = (x[b].reshape(ND, 128, NT, 128).transpose(2, 1, 0, 3)
              .reshape(NT, 128, ND * 128).copy())
        # w cols: [k h0..7 | v h0..7 | q h0..7] * 64
        a = qkv[:, hb:hb + 8]                 # [3(q,k,v), 8, D, 64]
        wp = np.empty((D, 24, DH), np.float32)
        wp[:, 0:8] = a[1].transpose(1, 0, 2)
        wp[:, 8:16] = a[2].transpose(1, 0, 2)
        wp[:, 16:24] = a[0].transpose(1, 0, 2)
        wp = wp.reshape(D, NHL * 192).reshape(ND, 128, NHL * 192).copy()
        in_maps.append({"x_local": xp, "w_local": wp})
    return in_maps


def gather(results):
    out = np.empty((B, D, T), np.float32)
    for c in range(8):
        out[c // 2, (c % 2) * 512: (c % 2) * 512 + 512, :] = \
            results[c]["out_local"]
    return out


def kernel(**inputs):
    from concourse.bass_utils import run_bass_kernel_spmd

    nc = _get_prog()
    in_maps = make_in_maps(inputs["x"], inputs["qkv"])
    res = run_bass_kernel_spmd(nc, in_maps, list(range(8)))
    return gather(res.results)
